# revision 1
# baseline (speedup 1.0000x reference)
"""Trainium2 Bass kernel for nn_ChebySemi (Chebyshev semi-iteration with
per-sample 3x3 stencil conv + power iteration), data-parallel over 8 cores.

Algorithm per sample (matches reference.py):
  power: 20x { y = conv3x3(pad(u)); m = max|y|; u = y/m }   -> m
  taus[k] = (1/m) * 2/(1.5 + 0.5*root_k)
  cheb:  15x { x += tau_k*(f - conv3x3(pad(x))) }

Mapping per core (8 samples):
  - padded image P [514,514] stored as 5 row-chunks [128 part, 5, 514] per
    sample; chunk c holds P rows [126c, 126c+128) (c<4), chunk 4 holds
    P rows [504,514) on partitions 0..9.
  - conv = banded matmuls (float32r): for each chunk, 3 col-shifted matmuls
    accumulate in PSUM; band S[k,p] = K[k-p+1,b] (k-p in {-1,0,1}, col p=0
    zeroed) so psum partition p aligns with U partition p.
  - per-step normalization: DVE abs-max reduces + cross-partition max via
    small transpose DMAs; ACT does the scaled PSUM->SBUF copy.
  - cheb update: ACT z=tau*f (bf16), GPSIMD U+=z, DVE U+=(-tau)*psum.
  - halo rows between chunks refreshed by 2 SBUF-SBUF DMAs per conv.
"""
import numpy as np
import ml_dtypes

B = 64
NCORES = 8
SPC = B // NCORES          # samples per core
M = 512
PW = 514
CH = 5                     # row chunks (4 main + 1 tail)
MMO = 127                  # matmul M (psum partitions; col 0 of band zeroed)
SW = CH * PW               # per-sample free width in U/F
NPOW = 20
NCHEB = 15
ALPHA = 0.5
ROOTS = np.cos(np.pi * (2 * np.arange(NCHEB) + 1) / (2 * NCHEB)).astype(np.float64)

_COMPILED = None


def _quant11(x):
    """Round fp32 to 11-bit mantissa (float32r input rounding)."""
    xi = np.ascontiguousarray(x, np.float32).view(np.uint32)
    shift = 23 - 11
    rb = np.uint32(1 << (shift - 1))
    mask = np.uint32(~((1 << shift) - 1) & 0xFFFFFFFF)
    return ((xi + rb) & mask).view(np.float32)


def _pad_layout(imgs, ones_pad):
    """imgs [N, 512, 512] -> [N, 128, 5, 514] chunk layout of padded P."""
    n = imgs.shape[0]
    P = np.zeros((n, PW, PW), np.float32)
    P[:, 1:513, 1:513] = imgs
    if ones_pad:
        P[:, 513, :] = 1.0
        P[:, :, 513] = 1.0
    out = np.zeros((n, 128, CH, PW), np.float32)
    for c in range(4):
        out[:, :, c, :] = P[:, 126 * c:126 * c + 128, :]
    out[:, 0:10, 4, :] = P[:, 504:514, :]
    return out


def _bands(kern):
    """kern [N,3,3] -> [N, 128, 3*127] shifted bands, col p=0 zeroed."""
    n = kern.shape[0]
    S = np.zeros((n, 128, 3 * MMO), np.float32)
    for b in range(3):
        for a in range(3):
            # S[:, k, b*127+p] = K[a, b] where k = p - 1 + a, p in [1,127)
            p = np.arange(1, MMO)
            k = p - 1 + a
            ok = (k >= 0) & (k < 128)
            S[:, k[ok], b * MMO + p[ok]] = kern[:, a, b][:, None]
    return S


def _build_program():
    import concourse.bass as bass
    import concourse.tile as tile
    from concourse import mybir, bacc
    from contextlib import ExitStack

    F32 = mybir.dt.float32
    F32R = mybir.dt.float32r
    BF16 = mybir.dt.bfloat16
    AX = mybir.AxisListType
    OP = mybir.AluOpType

    nc = bacc.Bacc("TRN2", target_bir_lowering=False, debug=False)

    u0p_d = nc.dram_tensor("u0p", [128, SPC * SW], F32, kind="ExternalInput")
    xp_d = nc.dram_tensor("xp", [128, SPC * SW], F32, kind="ExternalInput")
    fp_d = nc.dram_tensor("fp", [128, SPC * SW], BF16, kind="ExternalInput")
    wb_d = nc.dram_tensor("wb", [128, SPC * 3 * MMO], F32, kind="ExternalInput")
    c2q_d = nc.dram_tensor("c2q", [128, NCHEB], F32, kind="ExternalInput")
    nc2q_d = nc.dram_tensor("nc2q", [128, NCHEB], F32, kind="ExternalInput")
    out_d = nc.dram_tensor("out", [SPC * M, M], F32, kind="ExternalOutput")

    with tile.TileContext(nc) as tc, ExitStack() as ctx:
        sb = ctx.enter_context(tc.tile_pool(name="sb", bufs=1))
        ps = ctx.enter_context(tc.tile_pool(name="ps", bufs=3, space="PSUM"))
        p4p = ctx.enter_context(tc.tile_pool(name="p4p", bufs=2, space="PSUM"))
        zp = ctx.enter_context(tc.tile_pool(name="zp", bufs=2))

        U = sb.tile([128, SPC * SW], F32)
        Fm = sb.tile([128, SPC * SW], BF16)
        W = sb.tile([128, SPC * 3 * MMO], F32)
        C2Q = sb.tile([128, NCHEB], F32)
        NC2Q = sb.tile([128, NCHEB], F32)
        TAU = sb.tile([128, SPC * NCHEB], F32)
        NTAU = sb.tile([128, SPC * NCHEB], F32)
        RED3 = sb.tile([128, SPC * 4], F32)
        REDT = sb.tile([1, SPC * 128], F32)
        M1 = sb.tile([1, SPC], F32)
        INV1 = sb.tile([1, SPC], F32)
        INVROW = sb.tile([1, SPC * 128], F32)
        INVB = sb.tile([128, SPC], F32)
        ONES1 = sb.tile([1, 128], F32)

        nc.sync.dma_start(U[:].bitcast(F32R), u0p_d.ap()[:, :].bitcast(F32R))
        nc.sync.dma_start(Fm[:], fp_d.ap()[:, :])
        nc.sync.dma_start(W[:].bitcast(F32R), wb_d.ap()[:, :].bitcast(F32R))
        nc.sync.dma_start(C2Q[:], c2q_d.ap()[:, :])
        nc.sync.dma_start(NC2Q[:], nc2q_d.ap()[:, :])
        nc.vector.memset(ONES1[0:1, :], 1.0)
        nc.vector.memset(RED3[:, :], 0.0)

        def us(s):
            return U[:, s * SW:(s + 1) * SW].rearrange("p (c w) -> p c w", c=CH)

        def conv(s):
            Us = us(s)
            Ws = W[:, s * 3 * MMO:(s + 1) * 3 * MMO]
            pm0 = ps.tile([128, 1024], F32, tag="pm")
            pm1 = ps.tile([128, 1024], F32, tag="pm")
            p4 = p4p.tile([128, 512], F32, tag="p4")
            for g, pt in ((0, pm0), (1, pm1)):
                for ci in range(2):
                    c = 2 * g + ci
                    for b in range(3):
                        nc.tensor.matmul(
                            pt[0:MMO, ci * 512:(ci + 1) * 512],
                            Ws[:, b * MMO:(b + 1) * MMO].bitcast(F32R),
                            Us[0:128, c, b:b + 512].bitcast(F32R),
                            start=(b == 0), stop=(b == 2))
            for b in range(3):
                nc.tensor.matmul(
                    p4[0:9, 0:512],
                    Ws[0:10, b * MMO:b * MMO + 9].bitcast(F32R),
                    Us[0:10, 4, b:b + 512].bitcast(F32R),
                    start=(b == 0), stop=(b == 2))
            return pm0, pm1, p4

        def reduce_inv(s, pm0, pm1, p4):
            c0 = 4 * s
            nc.vector.tensor_reduce(
                RED3[0:MMO, c0:c0 + 1],
                pm0[0:MMO, :].rearrange("p (c w) -> p c w", c=2),
                axis=AX.XY, op=OP.max, apply_absolute_value=True)
            nc.vector.tensor_reduce(
                RED3[0:MMO, c0 + 1:c0 + 2],
                pm1[0:MMO, :].rearrange("p (c w) -> p c w", c=2),
                axis=AX.XY, op=OP.max, apply_absolute_value=True)
            nc.vector.tensor_reduce(
                RED3[0:9, c0 + 2:c0 + 3], p4[0:9, 0:512],
                axis=AX.X, op=OP.max, apply_absolute_value=True)
            nc.vector.tensor_reduce(
                RED3[0:128, c0 + 3:c0 + 4], RED3[0:128, c0:c0 + 3],
                axis=AX.X, op=OP.max)
            nc.sync.dma_start(REDT[0:1, s * 128:(s + 1) * 128],
                              RED3[0:128, c0 + 3:c0 + 4])
            nc.vector.tensor_reduce(
                M1[0:1, s:s + 1], REDT[0:1, s * 128:(s + 1) * 128],
                axis=AX.X, op=OP.max)
            nc.vector.reciprocal(INV1[0:1, s:s + 1], M1[0:1, s:s + 1])
            nc.vector.tensor_scalar_mul(
                INVROW[0:1, s * 128:(s + 1) * 128], ONES1[0:1, :],
                INV1[0:1, s:s + 1])
            nc.sync.dma_start(INVB[0:128, s:s + 1],
                              INVROW[0:1, s * 128:(s + 1) * 128])

        def halos(s):
            Us = us(s)
            nc.sync.dma_start(Us[0:1, 1:5, :].bitcast(F32R),
                              Us[126:127, 0:4, :].bitcast(F32R))
            nc.sync.dma_start(Us[127:128, 0:4, :].bitcast(F32R),
                              Us[1:2, 1:5, :].bitcast(F32R))

        # ---- power phase ----
        for it in range(1, NPOW + 1):
            for s in range(SPC):
                pm0, pm1, p4 = conv(s)
                reduce_inv(s, pm0, pm1, p4)
                if it < NPOW:
                    Us = us(s)
                    for g, pt in ((0, pm0), (1, pm1)):
                        nc.scalar.mul(
                            Us[0:MMO, 2 * g:2 * g + 2, 1:513].bitcast(F32R),
                            pt[0:MMO, :].rearrange("p (c w) -> p c w", c=2),
                            INVB[0:MMO, s:s + 1])
                    nc.scalar.mul(Us[0:9, 4, 1:513].bitcast(F32R),
                                  p4[0:9, 0:512], INVB[0:9, s:s + 1])
                    halos(s)
                else:
                    nc.vector.tensor_scalar_mul(
                        TAU[:, s * NCHEB:(s + 1) * NCHEB], C2Q[:, :],
                        INVB[:, s:s + 1])
                    nc.vector.tensor_scalar_mul(
                        NTAU[:, s * NCHEB:(s + 1) * NCHEB], NC2Q[:, :],
                        INVB[:, s:s + 1])

        # ---- cheb phase ----
        nc.sync.dma_start(U[:].bitcast(F32R), xp_d.ap()[:, :].bitcast(F32R))
        from concourse import mybir as _mb
        for k in range(NCHEB):
            for s in range(SPC):
                pm0, pm1, p4 = conv(s)
                Us = us(s)
                z = zp.tile([128, SW], BF16, tag="z")
                nc.scalar.mul(z[:, :], Fm[:, s * SW:(s + 1) * SW],
                              TAU[:, s * NCHEB + k:s * NCHEB + k + 1])
                nc.gpsimd.tensor_tensor(
                    U[:, s * SW:(s + 1) * SW].bitcast(F32R),
                    U[:, s * SW:(s + 1) * SW], z[:, :], op=OP.add)
                for g, pt in ((0, pm0), (1, pm1)):
                    nc.vector.scalar_tensor_tensor(
                        Us[0:MMO, 2 * g:2 * g + 2, 1:513].bitcast(F32R),
                        pt[0:MMO, :].rearrange("p (c w) -> p c w", c=2),
                        NTAU[0:MMO, s * NCHEB + k:s * NCHEB + k + 1],
                        Us[0:MMO, 2 * g:2 * g + 2, 1:513],
                        op0=OP.mult, op1=OP.add)
                nc.vector.scalar_tensor_tensor(
                    Us[0:9, 4, 1:513].bitcast(F32R), p4[0:9, 0:512],
                    NTAU[0:9, s * NCHEB + k:s * NCHEB + k + 1],
                    Us[0:9, 4, 1:513], op0=OP.mult, op1=OP.add)
                if k < NCHEB - 1:
                    halos(s)

        for s in range(SPC):
            Us = us(s)
            o = out_d.ap()[s * M:(s + 1) * M, :]
            nc.sync.dma_start(
                o[0:504, :].rearrange("(c p) w -> p c w", p=126),
                Us[1:MMO, 0:4, 1:513])
            nc.sync.dma_start(o[504:512, :], Us[1:9, 4, 1:513])

    nc.compile()
    return nc


def _prep_core_inputs(x, f, kernelA, u0):
    """Full [64,...] inputs -> list of 8 per-core input dicts."""
    x = np.asarray(x, np.float32).reshape(B, M, M)
    f = np.asarray(f, np.float32).reshape(B, M, M)
    kern = np.asarray(kernelA, np.float32).reshape(B, 3, 3)
    u0 = np.asarray(u0, np.float32).reshape(B, M, M)

    u0L = _quant11(_pad_layout(u0, True))     # [B,128,CH,PW]
    xL = _quant11(_pad_layout(x, True))
    fL = _pad_layout(f, False).astype(ml_dtypes.bfloat16)
    wbL = _quant11(_bands(kern))              # [B,128,381]

    c2q = (2.0 / (1.5 + 0.5 * ROOTS)).astype(np.float32)
    c2qT = np.broadcast_to(c2q, (128, NCHEB)).copy()
    nc2qT = (-c2qT).copy()

    in_maps = []
    for c in range(NCORES):
        sl = slice(c * SPC, (c + 1) * SPC)
        in_maps.append({
            "u0p": u0L[sl].transpose(1, 0, 2, 3).reshape(128, SPC * SW).copy(),
            "xp": xL[sl].transpose(1, 0, 2, 3).reshape(128, SPC * SW).copy(),
            "fp": fL[sl].transpose(1, 0, 2, 3).reshape(128, SPC * SW).copy(),
            "wb": wbL[sl].transpose(1, 0, 2).reshape(128, SPC * 3 * MMO).copy(),
            "c2q": c2qT,
            "nc2q": nc2qT,
        })
    return in_maps


def kernel(x, f, kernelA, u0):
    global _COMPILED
    from concourse import bass_utils

    if _COMPILED is None:
        _COMPILED = _build_program()
    nc = _COMPILED

    in_maps = _prep_core_inputs(x, f, kernelA, u0)
    res = bass_utils.run_bass_kernel_spmd(nc, in_maps, core_ids=list(range(NCORES)))
    out = np.stack([res.results[c]["out"] for c in range(NCORES)])  # [8, SPC*M, M]
    return out.reshape(B, 1, M, M).astype(np.float32)



# revision 6
# speedup vs baseline: 1.2181x; 1.2181x over previous
"""Trainium2 Bass kernel for nn_ChebySemi (Chebyshev semi-iteration with
per-sample 3x3 stencil conv + power iteration), data-parallel over 8 cores.

Algorithm per sample (matches reference.py):
  power: 20x { y = conv3x3(pad(u)); m = max|y|; u = y/m }   -> m
  cheb:  15x { x += tau_k*(f - conv3x3(pad(x))) },  tau_k = c2q_k/m

Key restructuring vs the straightforward mapping:
  - Power phase scales the BAND WEIGHTS by 1/m_t (and writes the right
    border column as m_t; the bottom border row stays 1 and its band row
    stays unscaled) instead of scaling the image, so the psum->SBUF copy
    does not wait on the max chain.
  - Cross-partition max via gpsimd partition_all_reduce, in two 4-sample
    groups so group A's 1/m chain hides under group B's convs.
  - Cheb phase pre-scales the band by -1/m and f by 1/m once; f/m is
    injected into PSUM by an extra identity matmul, so each update is a
    single DVE scalar_tensor_tensor with compile-time constant c2q_k.
  - In cheb, rows 504..513 of all 8 samples are batched into one
    128-partition "strip" (partition 16s+d = sample s padded row 504+d),
    so the ragged tail costs 4 matmuls per iteration instead of 32.
"""
import numpy as np
import ml_dtypes

B = 64
NCORES = 8
SPC = B // NCORES          # samples per core
M = 512
PW = 514
CH = 5                     # power row chunks (4 main + tail rows 504..513)
SW = CH * PW               # per-sample free width in U
SWF = 4 * PW               # per-sample free width in F (no tail chunk)
MB = 128                   # main band stationary width (cols 0,127 zero)
WBW = 3 * MB
TBW = 3 * 9                # tail band: 3 shifts x 9 cols (p=0..8, col0 zero)
NPOW = 20
NCHEB = 15
ROOTS = np.cos(np.pi * (2 * np.arange(NCHEB) + 1) / (2 * NCHEB)).astype(np.float64)
C2Q = [float(v) for v in (2.0 / (1.5 + 0.5 * ROOTS))]

_COMPILED = None


def _build_program():
    import concourse.bass as bass
    import concourse.tile as tile
    from concourse import mybir, bacc, bass_isa
    from contextlib import ExitStack

    F32 = mybir.dt.float32
    F32R = mybir.dt.float32r
    BF16 = mybir.dt.bfloat16
    AX = mybir.AxisListType
    OP = mybir.AluOpType
    ROP = bass_isa.ReduceOp

    nc = bacc.Bacc("TRN2", target_bir_lowering=False, debug=False)

    u0c_d = nc.dram_tensor("u0c", [128, SPC * SW], F32, kind="ExternalInput")
    xc_d = nc.dram_tensor("xc", [128, SPC * SWF], F32, kind="ExternalInput")
    fc_d = nc.dram_tensor("fc", [128, SPC * SWF], BF16, kind="ExternalInput")
    xs_d = nc.dram_tensor("xs", [128, PW], F32, kind="ExternalInput")
    fs_d = nc.dram_tensor("fs", [128, PW], BF16, kind="ExternalInput")
    wb_d = nc.dram_tensor("wb", [128, SPC * WBW], F32, kind="ExternalInput")
    wt5_d = nc.dram_tensor("wt5", [128, SPC * TBW], F32, kind="ExternalInput")
    wts_d = nc.dram_tensor("wts", [128, WBW], F32, kind="ExternalInput")
    iz_d = nc.dram_tensor("iz", [128, 128], BF16, kind="ExternalInput")
    izs_d = nc.dram_tensor("izs", [128, 128], BF16, kind="ExternalInput")
    out_d = nc.dram_tensor("out", [SPC * M, M], F32, kind="ExternalOutput")

    with tile.TileContext(nc) as tc, ExitStack() as ctx:
        sb = ctx.enter_context(tc.tile_pool(name="sb", bufs=1))
        pm = ctx.enter_context(tc.tile_pool(name="pm", bufs=3, space="PSUM"))
        pst = ctx.enter_context(tc.tile_pool(name="pst", bufs=2, space="PSUM"))

        U = sb.tile([128, SPC * SW], F32)
        US = sb.tile([128, PW], F32)
        Fm = sb.tile([128, SPC * SWF], BF16)
        FS = sb.tile([128, PW], BF16)
        WB = sb.tile([128, SPC * WBW], F32)
        WP = sb.tile([128, SPC * WBW], F32)
        WT5 = sb.tile([128, SPC * TBW], F32)
        WT5P = sb.tile([128, SPC * TBW], F32)
        WTS = sb.tile([128, WBW], F32)
        WTSP = sb.tile([128, WBW], F32)
        IZ = sb.tile([128, 128], BF16)
        IZS = sb.tile([128, 128], BF16)
        RQ = sb.tile([128, 16], F32)    # 2 cols per sample: main-tile maxima
        RED = sb.tile([128, 16], F32)   # per group g: main 8g..+4, tail +4..+8
        MALL = sb.tile([128, 16], F32)
        M8 = sb.tile([128, 8], F32)
        INV8 = sb.tile([128, 8], F32)
        NINV8 = sb.tile([128, 8], F32)
        INV128 = sb.tile([128, 1], F32)
        NINV128 = sb.tile([128, 1], F32)
        ONES = sb.tile([128, PW], F32)

        nc.sync.dma_start(WB[:].bitcast(F32R), wb_d.ap()[:, :].bitcast(F32R))
        nc.sync.dma_start(WT5[:].bitcast(F32R), wt5_d.ap()[:, :].bitcast(F32R))
        nc.sync.dma_start(WTS[:].bitcast(F32R), wts_d.ap()[:, :].bitcast(F32R))
        nc.sync.dma_start(U[:].bitcast(F32R), u0c_d.ap()[:, :].bitcast(F32R))
        nc.sync.dma_start(IZ[:], iz_d.ap()[:, :])
        nc.sync.dma_start(IZS[:], izs_d.ap()[:, :])
        nc.sync.dma_start(Fm[:], fc_d.ap()[:, :])
        nc.sync.dma_start(FS[:], fs_d.ap()[:, :])
        nc.vector.memset(ONES[:, :], 1.0)
        nc.vector.tensor_copy(WP[:, :].bitcast(F32R), WB[:, :])
        nc.vector.tensor_copy(WT5P[:, :].bitcast(F32R), WT5[:, :])

        def us(s):
            return U[:, s * SW:(s + 1) * SW].rearrange("p (c w) -> p c w", c=CH)

        def fmv(s):
            return Fm[:, s * SWF:(s + 1) * SWF].rearrange("p (c w) -> p c w", c=4)

        def wp(s, b):
            o = s * WBW + b * MB
            return WP[:, o:o + MB]

        def conv_mms(s, cheb):
            """12 (power) / 16 (cheb) matmuls -> two [128,1024] psum tiles."""
            tiles = []
            for g in range(2):
                pt = pm.tile([128, 1024], F32, tag="pm")
                for ci in range(2):
                    c = 2 * g + ci
                    sl = pt[0:128, ci * 512:(ci + 1) * 512]
                    for b in range(3):
                        nc.tensor.matmul(
                            sl, wp(s, b).bitcast(F32R),
                            us(s)[0:128, c, b:b + 512].bitcast(F32R),
                            start=(b == 0), stop=(False if cheb else b == 2))
                    if cheb:
                        nc.tensor.matmul(sl, IZ[:, :],
                                         fmv(s)[0:128, c, 1:513],
                                         start=False, stop=True)
                tiles.append(pt)
            return tiles

        def tail_mms(s):
            pt = pst.tile([128, 512], F32, tag="ps")
            for b in range(3):
                o = s * TBW + b * 9
                nc.tensor.matmul(
                    pt[0:9, :], WT5P[0:10, o:o + 9].bitcast(F32R),
                    us(s)[0:10, 4, b:b + 512].bitcast(F32R),
                    start=(b == 0), stop=(b == 2))
            return pt

        def strip_mms():
            pt = pst.tile([128, 512], F32, tag="ps")
            for b in range(3):
                nc.tensor.matmul(
                    pt[0:128, :], WTSP[:, b * MB:(b + 1) * MB].bitcast(F32R),
                    US[0:128, b:b + 512].bitcast(F32R),
                    start=(b == 0), stop=False)
            nc.tensor.matmul(pt[0:128, :], IZS[:, :], FS[0:128, 1:513],
                             start=False, stop=True)
            return pt

        def halos(s):
            Us = us(s)
            nc.sync.dma_start(Us[0:1, 1:5, :].bitcast(F32R),
                              Us[126:127, 0:4, :].bitcast(F32R))
            nc.sync.dma_start(Us[127:128, 0:4, :].bitcast(F32R),
                              Us[1:2, 1:5, :].bitcast(F32R))

        def cheb_halos(s):
            Us = us(s)
            nc.sync.dma_start(Us[0:1, 1:4, :].bitcast(F32R),
                              Us[126:127, 0:3, :].bitcast(F32R))
            nc.sync.dma_start(Us[127:128, 0:3, :].bitcast(F32R),
                              Us[1:2, 1:4, :].bitcast(F32R))

        def strip_halos():
            # strip row 16s+0 (P row 504) <- chunk3 partition 126
            # chunk3 partition 127 (P row 505) <- strip row 16s+1
            for s in range(SPC):
                o = s * SW + 3 * PW
                nc.sync.dma_start(US[16 * s:16 * s + 1, :].bitcast(F32R),
                                  U[126:127, o:o + PW].bitcast(F32R))
                nc.sync.dma_start(U[127:128, o:o + PW].bitcast(F32R),
                                  US[16 * s + 1:16 * s + 2, :].bitcast(F32R))

        # ================= power phase =================
        for t in range(1, NPOW + 1):
            last = (t == NPOW)
            for g in range(2):
                c0 = 8 * g
                nc.gpsimd.memset(RED[:, c0 + 4:c0 + 8], 0.0)
                for i, s in enumerate(range(4 * g, 4 * g + 4)):
                    ta, tb = conv_mms(s, cheb=False)
                    p4 = tail_mms(s)
                    nc.vector.tensor_reduce(
                        RQ[:, 2 * s:2 * s + 1],
                        ta[0:128, :].rearrange("p (c w) -> p c w", c=2),
                        axis=AX.XY, op=OP.max, apply_absolute_value=True)
                    nc.vector.tensor_reduce(
                        RQ[:, 2 * s + 1:2 * s + 2],
                        tb[0:128, :].rearrange("p (c w) -> p c w", c=2),
                        axis=AX.XY, op=OP.max, apply_absolute_value=True)
                    nc.vector.tensor_reduce(
                        RED[0:9, c0 + 4 + i:c0 + 5 + i], p4[0:9, :],
                        axis=AX.X, op=OP.max, apply_absolute_value=True)
                    if not last:
                        for gg, pt in ((0, ta), (1, tb)):
                            nc.scalar.copy(
                                us(s)[0:127, 2 * gg:2 * gg + 2, 1:513]
                                .bitcast(F32R),
                                pt[0:127, :].rearrange("p (c w) -> p c w", c=2))
                        nc.scalar.copy(us(s)[0:9, 4, 1:513].bitcast(F32R),
                                       p4[0:9, :])
                        halos(s)
                # m chain for this group
                nc.vector.tensor_reduce(
                    RED[:, c0:c0 + 4],
                    RQ[:, c0:c0 + 8].rearrange("p (s q) -> p s q", q=2),
                    axis=AX.X, op=OP.max)
                nc.gpsimd.partition_all_reduce(
                    MALL[:, c0:c0 + 8], RED[:, c0:c0 + 8], 128, ROP.max)
                nc.vector.tensor_tensor(
                    M8[:, 4 * g:4 * g + 4], MALL[:, c0:c0 + 4],
                    MALL[:, c0 + 4:c0 + 8], op=OP.max)
                nc.vector.reciprocal(INV8[:, 4 * g:4 * g + 4],
                                     M8[:, 4 * g:4 * g + 4])
                if not last:
                    for s in range(4 * g, 4 * g + 4):
                        iv = INV8[:, s:s + 1]
                        nc.gpsimd.tensor_scalar_mul(
                            WP[:, s * WBW:(s + 1) * WBW].bitcast(F32R),
                            WB[:, s * WBW:(s + 1) * WBW], iv)
                        # tail band rows 0..8 scale; row 9 (reads the
                        # static all-ones bottom border row) stays raw
                        nc.gpsimd.tensor_scalar_mul(
                            WT5P[0:9, s * TBW:(s + 1) * TBW].bitcast(F32R),
                            WT5[0:9, s * TBW:(s + 1) * TBW], INV8[0:9, s:s + 1])
                        # right border col <- m_t (chunks 0..3 all rows;
                        # chunk4 rows 504..512 only — corner (513,513) stays 1)
                        nc.gpsimd.tensor_scalar_mul(
                            us(s)[0:128, 0:4, 513:514]
                            .rearrange("p c w -> p (c w)").bitcast(F32R),
                            ONES[:, 0:4], M8[:, s:s + 1])
                        nc.gpsimd.tensor_scalar_mul(
                            us(s)[0:9, 4, 513:514].bitcast(F32R),
                            ONES[0:9, 0:1], M8[0:9, s:s + 1])

        # ================= cheb setup =================
        for s in range(SPC):
            nc.sync.dma_start(
                U[:, s * SW:s * SW + SWF].bitcast(F32R),
                xc_d.ap()[:, s * SWF:(s + 1) * SWF].bitcast(F32R))
        nc.sync.dma_start(US[:].bitcast(F32R), xs_d.ap()[:, :].bitcast(F32R))
        nc.vector.tensor_scalar_mul(NINV8[:, :], INV8[:, :], -1.0)
        for s in range(SPC):
            nc.sync.dma_start(INV128[16 * s:16 * s + 16, 0:1],
                              INV8[16 * s:16 * s + 16, s:s + 1])
        nc.vector.tensor_scalar_mul(NINV128[:, :], INV128[:, :], -1.0)
        for s in range(SPC):
            nc.gpsimd.tensor_scalar_mul(
                WP[:, s * WBW:(s + 1) * WBW].bitcast(F32R),
                WB[:, s * WBW:(s + 1) * WBW], NINV8[:, s:s + 1])
            nc.gpsimd.tensor_scalar_mul(
                WT5P[0:10, s * TBW:(s + 1) * TBW].bitcast(F32R),
                WT5[0:10, s * TBW:(s + 1) * TBW], NINV8[0:10, s:s + 1])
            eng = nc.gpsimd if s % 2 == 0 else nc.vector
            eng.tensor_scalar_mul(Fm[:, s * SWF:(s + 1) * SWF],
                                  Fm[:, s * SWF:(s + 1) * SWF],
                                  INV8[:, s:s + 1])
        nc.gpsimd.tensor_scalar_mul(WTSP[:, :].bitcast(F32R), WTS[:, :],
                                    NINV128[:, 0:1])
        nc.gpsimd.tensor_scalar_mul(FS[:, :], FS[:, :], INV128[:, 0:1])

        # ================= cheb phase =================
        for k in range(NCHEB):
            ck = C2Q[k]
            for s in range(SPC):
                ta, tb = conv_mms(s, cheb=True)
                for gg, pt in ((0, ta), (1, tb)):
                    sl = us(s)[0:127, 2 * gg:2 * gg + 2, 1:513]
                    nc.vector.scalar_tensor_tensor(
                        sl.bitcast(F32R),
                        pt[0:127, :].rearrange("p (c w) -> p c w", c=2),
                        ck, sl, op0=OP.mult, op1=OP.add)
                if k < NCHEB - 1:
                    cheb_halos(s)
            ps = strip_mms()
            nc.vector.scalar_tensor_tensor(
                US[0:128, 1:513].bitcast(F32R), ps[0:128, :], ck,
                US[0:128, 1:513], op0=OP.mult, op1=OP.add)
            if k < NCHEB - 1:
                strip_halos()

        for s in range(SPC):
            o = out_d.ap()[s * M:(s + 1) * M, :]
            nc.sync.dma_start(
                o[0:504, :].rearrange("(c p) w -> p c w", p=126),
                us(s)[1:127, 0:4, 1:513])
            nc.sync.dma_start(o[504:512, :], US[16 * s + 1:16 * s + 9, 1:513])

    nc.compile()
    return nc


def _prep_core_inputs(x, f, kernelA, u0):
    """Full [64,...] inputs -> list of 8 per-core input dicts."""
    x = np.asarray(x, np.float32).reshape(B, M, M)
    f = np.asarray(f, np.float32).reshape(B, M, M)
    kern = np.asarray(kernelA, np.float32).reshape(B, 3, 3)
    u0 = np.asarray(u0, np.float32).reshape(B, M, M)

    def pad_full(img, ones_pad):
        P = np.zeros((B, PW, PW), np.float32)
        P[:, 1:513, 1:513] = img
        if ones_pad:
            P[:, 513, :] = 1.0
            P[:, :, 513] = 1.0
        return P

    def chunks5(P):
        Z = np.zeros((B, 128, CH, PW), np.float32)
        for c in range(4):
            Z[:, :, c, :] = P[:, 126 * c:126 * c + 128, :]
        Z[:, 0:10, 4, :] = P[:, 504:514, :]
        return Z

    def chunks4(P):
        Z = np.zeros((B, 128, 4, PW), np.float32)
        for c in range(4):
            Z[:, :, c, :] = P[:, 126 * c:126 * c + 128, :]
        return Z

    Pu = pad_full(u0, True)
    Px = pad_full(x, True)
    Pf = pad_full(f, False)
    u0cL = chunks5(Pu)
    xcL = chunks4(Px)
    fcL = chunks4(Pf)

    # cheb strip: partition 16s+d = sample s P row 504+d
    xsL = np.zeros((NCORES, 128, PW), np.float32)
    fsL = np.zeros((NCORES, 128, PW), np.float32)
    for co in range(NCORES):
        for s in range(SPC):
            xsL[co, 16 * s:16 * s + 10, :] = Px[co * SPC + s, 504:514, :]
            fsL[co, 16 * s + 1:16 * s + 9, :] = Pf[co * SPC + s, 505:513, :]

    # main band: S[k = p-1+a, b*MB+p] = K[a,b], p in 1..126
    wbL = np.zeros((B, 128, WBW), np.float32)
    p_idx = np.arange(1, 127)
    for b in range(3):
        for a in range(3):
            wbL[:, p_idx - 1 + a, b * MB + p_idx] = kern[:, a, b][:, None]
    # per-sample tail band: S[p-1+a, b*9+p] = K[a,b], p in 1..8 (rows 0..9)
    wt5L = np.zeros((B, 128, TBW), np.float32)
    pt_idx = np.arange(1, 9)
    for b in range(3):
        for a in range(3):
            wt5L[:, pt_idx - 1 + a, b * 9 + pt_idx] = kern[:, a, b][:, None]
    # cheb strip band: S[16s+p-1+a, b*MB+16s+p] = K_s[a,b], p in 1..8
    wtsL = np.zeros((NCORES, 128, WBW), np.float32)
    for co in range(NCORES):
        for s in range(SPC):
            ks = kern[co * SPC + s]
            for b in range(3):
                for a in range(3):
                    wtsL[co, 16 * s + pt_idx - 1 + a, b * MB + 16 * s + pt_idx] \
                        = ks[a, b]

    iz = np.zeros((128, 128), np.float32)
    iz[np.arange(1, 127), np.arange(1, 127)] = 1.0
    izs = np.eye(128, dtype=np.float32)
    izL = iz.astype(ml_dtypes.bfloat16)
    izsL = izs.astype(ml_dtypes.bfloat16)

    in_maps = []
    for co in range(NCORES):
        sl = slice(co * SPC, (co + 1) * SPC)
        in_maps.append({
            "u0c": u0cL[sl].transpose(1, 0, 2, 3).reshape(128, SPC * SW).copy(),
            "xc": xcL[sl].transpose(1, 0, 2, 3).reshape(128, SPC * SWF).copy(),
            "fc": fcL[sl].transpose(1, 0, 2, 3).reshape(128, SPC * SWF)
                  .astype(ml_dtypes.bfloat16),
            "xs": xsL[co].copy(),
            "fs": fsL[co].astype(ml_dtypes.bfloat16),
            "wb": wbL[sl].transpose(1, 0, 2).reshape(128, SPC * WBW).copy(),
            "wt5": wt5L[sl].transpose(1, 0, 2).reshape(128, SPC * TBW).copy(),
            "wts": wtsL[co].copy(),
            "iz": izL,
            "izs": izsL,
        })
    return in_maps


def kernel(x, f, kernelA, u0):
    global _COMPILED
    from concourse import bass_utils

    if _COMPILED is None:
        _COMPILED = _build_program()
    nc = _COMPILED

    in_maps = _prep_core_inputs(x, f, kernelA, u0)
    res = bass_utils.run_bass_kernel_spmd(nc, in_maps, core_ids=list(range(NCORES)))
    out = np.stack([res.results[c]["out"] for c in range(NCORES)])  # [8, SPC*M, M]
    return out.reshape(B, 1, M, M).astype(np.float32)


# revision 8
# speedup vs baseline: 1.5643x; 1.2842x over previous
"""Trainium2 Bass kernel for nn_ChebySemi (Chebyshev semi-iteration with
per-sample 3x3 stencil conv + power iteration), data-parallel over 8 cores.

Algorithm per sample (matches reference.py):
  power: 20x { y = conv3x3(pad(u)); m = max|y|; u = y/m }   -> m
  cheb:  15x { x += tau_k*(f - conv3x3(pad(x))) },  tau_k = c2q_k/m

Key restructuring vs the straightforward mapping:
  - Power phase scales the BAND WEIGHTS by 1/m_t (and writes the right
    border column as m_t; the bottom border row stays 1 and its band row
    stays unscaled) instead of scaling the image, so the psum->SBUF copy
    does not wait on the max chain.
  - Cross-partition max via gpsimd partition_all_reduce, in two 4-sample
    groups so group A's 1/m chain hides under group B's convs.
  - Cheb phase pre-scales the band by -1/m and f by 1/m once; f/m is
    injected into PSUM by an extra identity matmul, so each update is a
    single DVE scalar_tensor_tensor with compile-time constant c2q_k.
  - In cheb, rows 504..513 of all 8 samples are batched into one
    128-partition "strip" (partition 16s+d = sample s padded row 504+d),
    so the ragged tail costs 4 matmuls per iteration instead of 32.
"""
import numpy as np
import ml_dtypes

B = 64
NCORES = 8
SPC = B // NCORES          # samples per core
M = 512
PW = 514
CH = 5                     # power row chunks (4 main + tail rows 504..513)
SW = CH * PW               # per-sample free width in U
SWF = 4 * PW               # per-sample free width in F (no tail chunk)
MB = 128                   # main band stationary width (cols 0,127 zero)
WBW = 3 * MB
TBW = 3 * 9                # tail band: 3 shifts x 9 cols (p=0..8, col0 zero)
NPOW = 20
NCHEB = 15
ROOTS = np.cos(np.pi * (2 * np.arange(NCHEB) + 1) / (2 * NCHEB)).astype(np.float64)
C2Q = [float(v) for v in (2.0 / (1.5 + 0.5 * ROOTS))]

_COMPILED = None


def _build_program():
    import concourse.bass as bass
    import concourse.tile as tile
    from concourse import mybir, bacc, bass_isa
    from contextlib import ExitStack

    F32 = mybir.dt.float32
    F32R = mybir.dt.float32r
    BF16 = mybir.dt.bfloat16
    AX = mybir.AxisListType
    OP = mybir.AluOpType
    ROP = bass_isa.ReduceOp

    nc = bacc.Bacc("TRN2", target_bir_lowering=False, debug=False)

    u0c_d = nc.dram_tensor("u0c", [128, SPC * SW], F32, kind="ExternalInput")
    xc_d = nc.dram_tensor("xc", [128, SPC * SWF], F32, kind="ExternalInput")
    fc_d = nc.dram_tensor("fc", [128, SPC * SWF], BF16, kind="ExternalInput")
    xs_d = nc.dram_tensor("xs", [128, PW], F32, kind="ExternalInput")
    fs_d = nc.dram_tensor("fs", [128, PW], BF16, kind="ExternalInput")
    wb_d = nc.dram_tensor("wb", [128, SPC * WBW], F32, kind="ExternalInput")
    wt5_d = nc.dram_tensor("wt5", [128, SPC * TBW], F32, kind="ExternalInput")
    wts_d = nc.dram_tensor("wts", [128, WBW], F32, kind="ExternalInput")
    iz_d = nc.dram_tensor("iz", [128, 128], BF16, kind="ExternalInput")
    izs_d = nc.dram_tensor("izs", [128, 128], BF16, kind="ExternalInput")
    out_d = nc.dram_tensor("out", [SPC * M, M], F32, kind="ExternalOutput")

    with tile.TileContext(nc) as tc, ExitStack() as ctx:
        sb = ctx.enter_context(tc.tile_pool(name="sb", bufs=1))
        pm = ctx.enter_context(tc.tile_pool(name="pm", bufs=3, space="PSUM"))
        pst = ctx.enter_context(tc.tile_pool(name="pst", bufs=2, space="PSUM"))
        zp = ctx.enter_context(tc.tile_pool(name="zp", bufs=2))

        U = sb.tile([128, SPC * SW], F32)
        US = sb.tile([128, PW], F32)
        Fm = sb.tile([128, SPC * SWF], BF16)
        FS = sb.tile([128, PW], BF16)
        WB = sb.tile([128, SPC * WBW], F32)
        WP = sb.tile([128, SPC * WBW], F32)
        WT5 = sb.tile([128, SPC * TBW], F32)
        WT5P = sb.tile([128, SPC * TBW], F32)
        WTS = sb.tile([128, WBW], F32)
        WTSP = sb.tile([128, WBW], F32)
        IZ = sb.tile([128, 128], BF16)
        IZS = sb.tile([128, 128], BF16)
        RQ = sb.tile([128, 16], F32)    # 2 cols per sample: main-tile maxima
        RED = sb.tile([128, 16], F32)   # per group g: main 8g..+4, tail +4..+8
        MALL = sb.tile([128, 16], F32)
        M8 = sb.tile([128, 8], F32)
        INV8 = sb.tile([128, 8], F32)
        NINV8 = sb.tile([128, 8], F32)
        INV128 = sb.tile([128, 1], F32)
        NINV128 = sb.tile([128, 1], F32)
        ONES = sb.tile([128, PW], F32)

        nc.sync.dma_start(WB[:].bitcast(F32R), wb_d.ap()[:, :].bitcast(F32R))
        nc.sync.dma_start(WT5[:].bitcast(F32R), wt5_d.ap()[:, :].bitcast(F32R))
        nc.sync.dma_start(WTS[:].bitcast(F32R), wts_d.ap()[:, :].bitcast(F32R))
        nc.sync.dma_start(U[:].bitcast(F32R), u0c_d.ap()[:, :].bitcast(F32R))
        nc.sync.dma_start(IZ[:], iz_d.ap()[:, :])
        nc.sync.dma_start(IZS[:], izs_d.ap()[:, :])
        nc.sync.dma_start(Fm[:], fc_d.ap()[:, :])
        nc.sync.dma_start(FS[:], fs_d.ap()[:, :])
        nc.vector.memset(ONES[:, :], 1.0)
        nc.vector.tensor_copy(WP[:, :].bitcast(F32R), WB[:, :])
        nc.vector.tensor_copy(WT5P[:, :].bitcast(F32R), WT5[:, :])

        def us(s):
            return U[:, s * SW:(s + 1) * SW].rearrange("p (c w) -> p c w", c=CH)

        def fmv(s):
            return Fm[:, s * SWF:(s + 1) * SWF].rearrange("p (c w) -> p c w", c=4)

        def wp(s, b):
            o = s * WBW + b * MB
            return WP[:, o:o + MB]

        def conv_mms(s, inject):
            """12 (+4 if inject) matmuls -> two [128,1024] psum tiles."""
            tiles = []
            for g in range(2):
                pt = pm.tile([128, 1024], F32, tag="pm")
                for ci in range(2):
                    c = 2 * g + ci
                    sl = pt[0:128, ci * 512:(ci + 1) * 512]
                    for b in range(3):
                        nc.tensor.matmul(
                            sl, wp(s, b).bitcast(F32R),
                            us(s)[0:128, c, b:b + 512].bitcast(F32R),
                            start=(b == 0), stop=(False if inject else b == 2))
                    if inject:
                        nc.tensor.matmul(sl, IZ[:, :],
                                         fmv(s)[0:128, c, 1:513],
                                         start=False, stop=True)
                tiles.append(pt)
            return tiles

        def tail_mms(s):
            pt = pst.tile([128, 512], F32, tag="ps")
            for b in range(3):
                o = s * TBW + b * 9
                nc.tensor.matmul(
                    pt[0:9, :], WT5P[0:10, o:o + 9].bitcast(F32R),
                    us(s)[0:10, 4, b:b + 512].bitcast(F32R),
                    start=(b == 0), stop=(b == 2))
            return pt

        def strip_mms():
            pt = pst.tile([128, 512], F32, tag="ps")
            for b in range(3):
                nc.tensor.matmul(
                    pt[0:128, :], WTSP[:, b * MB:(b + 1) * MB].bitcast(F32R),
                    US[0:128, b:b + 512].bitcast(F32R),
                    start=(b == 0), stop=False)
            nc.tensor.matmul(pt[0:128, :], IZS[:, :], FS[0:128, 1:513],
                             start=False, stop=True)
            return pt

        def halos(s):
            Us = us(s)
            nc.sync.dma_start(Us[0:1, 1:5, :].bitcast(F32R),
                              Us[126:127, 0:4, :].bitcast(F32R))
            nc.sync.dma_start(Us[127:128, 0:4, :].bitcast(F32R),
                              Us[1:2, 1:5, :].bitcast(F32R))

        def cheb_halos(s):
            Us = us(s)
            nc.sync.dma_start(Us[0:1, 1:4, :].bitcast(F32R),
                              Us[126:127, 0:3, :].bitcast(F32R))
            nc.sync.dma_start(Us[127:128, 0:3, :].bitcast(F32R),
                              Us[1:2, 1:4, :].bitcast(F32R))

        def strip_halos():
            # strip row 16s+0 (P row 504) <- chunk3 partition 126
            # chunk3 partition 127 (P row 505) <- strip row 16s+1
            for s in range(SPC):
                o = s * SW + 3 * PW
                nc.sync.dma_start(US[16 * s:16 * s + 1, :].bitcast(F32R),
                                  U[126:127, o:o + PW].bitcast(F32R))
                nc.sync.dma_start(U[127:128, o:o + PW].bitcast(F32R),
                                  US[16 * s + 1:16 * s + 2, :].bitcast(F32R))

        # ================= power phase =================
        for t in range(1, NPOW + 1):
            last = (t == NPOW)
            for g in range(2):
                c0 = 8 * g
                nc.gpsimd.memset(RED[:, c0 + 4:c0 + 8], 0.0)
                for i, s in enumerate(range(4 * g, 4 * g + 4)):
                    ta, tb = conv_mms(s, inject=False)
                    p4 = tail_mms(s)
                    nc.vector.tensor_reduce(
                        RQ[:, 2 * s:2 * s + 1],
                        ta[0:128, :].rearrange("p (c w) -> p c w", c=2),
                        axis=AX.XY, op=OP.max, apply_absolute_value=True)
                    nc.vector.tensor_reduce(
                        RQ[:, 2 * s + 1:2 * s + 2],
                        tb[0:128, :].rearrange("p (c w) -> p c w", c=2),
                        axis=AX.XY, op=OP.max, apply_absolute_value=True)
                    nc.vector.tensor_reduce(
                        RED[0:9, c0 + 4 + i:c0 + 5 + i], p4[0:9, :],
                        axis=AX.X, op=OP.max, apply_absolute_value=True)
                    if not last:
                        for gg, pt in ((0, ta), (1, tb)):
                            nc.scalar.copy(
                                us(s)[0:127, 2 * gg:2 * gg + 2, 1:513]
                                .bitcast(F32R),
                                pt[0:127, :].rearrange("p (c w) -> p c w", c=2))
                        nc.scalar.copy(us(s)[0:9, 4, 1:513].bitcast(F32R),
                                       p4[0:9, :])
                        halos(s)
                # m chain for this group
                nc.vector.tensor_reduce(
                    RED[:, c0:c0 + 4],
                    RQ[:, c0:c0 + 8].rearrange("p (s q) -> p s q", q=2),
                    axis=AX.X, op=OP.max)
                nc.gpsimd.partition_all_reduce(
                    MALL[:, c0:c0 + 8], RED[:, c0:c0 + 8], 128, ROP.max)
                nc.vector.tensor_tensor(
                    M8[:, 4 * g:4 * g + 4], MALL[:, c0:c0 + 4],
                    MALL[:, c0 + 4:c0 + 8], op=OP.max)
                nc.vector.reciprocal(INV8[:, 4 * g:4 * g + 4],
                                     M8[:, 4 * g:4 * g + 4])
                if not last:
                    for s in range(4 * g, 4 * g + 4):
                        iv = INV8[:, s:s + 1]
                        nc.scalar.mul(
                            WP[:, s * WBW:(s + 1) * WBW].bitcast(F32R),
                            WB[:, s * WBW:(s + 1) * WBW], iv)
                        # tail band rows 0..8 scale; row 9 (reads the
                        # static all-ones bottom border row) stays raw
                        nc.vector.tensor_scalar_mul(
                            WT5P[0:9, s * TBW:(s + 1) * TBW].bitcast(F32R),
                            WT5[0:9, s * TBW:(s + 1) * TBW], INV8[0:9, s:s + 1])
                        # right border col <- m_t (chunks 0..3 all rows;
                        # chunk4 rows 504..512 only — corner (513,513) stays 1)
                        nc.scalar.mul(
                            us(s)[0:128, 0:4, 513:514]
                            .rearrange("p c w -> p (c w)").bitcast(F32R),
                            ONES[:, 0:4], M8[:, s:s + 1])
                        nc.vector.tensor_scalar_mul(
                            us(s)[0:9, 4, 513:514].bitcast(F32R),
                            ONES[0:9, 0:1], M8[0:9, s:s + 1])

        # ================= cheb setup =================
        for s in range(SPC):
            nc.sync.dma_start(
                U[:, s * SW:s * SW + SWF].bitcast(F32R),
                xc_d.ap()[:, s * SWF:(s + 1) * SWF].bitcast(F32R))
        nc.sync.dma_start(US[:].bitcast(F32R), xs_d.ap()[:, :].bitcast(F32R))
        nc.vector.tensor_scalar_mul(NINV8[:, :], INV8[:, :], -1.0)
        for s in range(SPC):
            nc.sync.dma_start(INV128[16 * s:16 * s + 16, 0:1],
                              INV8[16 * s:16 * s + 16, s:s + 1])
        nc.vector.tensor_scalar_mul(NINV128[:, :], INV128[:, :], -1.0)
        for s in range(SPC):
            nc.gpsimd.tensor_scalar_mul(
                WP[:, s * WBW:(s + 1) * WBW].bitcast(F32R),
                WB[:, s * WBW:(s + 1) * WBW], NINV8[:, s:s + 1])
            nc.gpsimd.tensor_scalar_mul(
                WT5P[0:10, s * TBW:(s + 1) * TBW].bitcast(F32R),
                WT5[0:10, s * TBW:(s + 1) * TBW], NINV8[0:10, s:s + 1])
            eng = nc.gpsimd if s % 2 == 0 else nc.vector
            eng.tensor_scalar_mul(Fm[:, s * SWF:(s + 1) * SWF],
                                  Fm[:, s * SWF:(s + 1) * SWF],
                                  INV8[:, s:s + 1])
        nc.gpsimd.tensor_scalar_mul(WTSP[:, :].bitcast(F32R), WTS[:, :],
                                    NINV128[:, 0:1])
        nc.gpsimd.tensor_scalar_mul(FS[:, :], FS[:, :], INV128[:, 0:1])

        # ================= cheb phase =================
        for k in range(NCHEB):
            ck = C2Q[k]
            for s in range(SPC):
                inject = s < 5
                ta, tb = conv_mms(s, inject=inject)
                for gg, pt in ((0, ta), (1, tb)):
                    sl = us(s)[0:127, 2 * gg:2 * gg + 2, 1:513]
                    nc.vector.scalar_tensor_tensor(
                        sl.bitcast(F32R),
                        pt[0:127, :].rearrange("p (c w) -> p c w", c=2),
                        ck, sl, op0=OP.mult, op1=OP.add)
                if not inject:
                    z = zp.tile([128, 2048], BF16, tag="z")
                    nc.scalar.mul(
                        z[0:127, :].rearrange("p (c w) -> p c w", c=4),
                        fmv(s)[0:127, :, 1:513], ck)
                    nc.gpsimd.tensor_tensor(
                        us(s)[0:127, 0:4, 1:513].bitcast(F32R),
                        us(s)[0:127, 0:4, 1:513],
                        z[0:127, :].rearrange("p (c w) -> p c w", c=4),
                        op=OP.add)
                if k < NCHEB - 1:
                    cheb_halos(s)
            ps = strip_mms()
            nc.vector.scalar_tensor_tensor(
                US[0:128, 1:513].bitcast(F32R), ps[0:128, :], ck,
                US[0:128, 1:513], op0=OP.mult, op1=OP.add)
            if k < NCHEB - 1:
                strip_halos()

        for s in range(SPC):
            o = out_d.ap()[s * M:(s + 1) * M, :]
            nc.sync.dma_start(
                o[0:504, :].rearrange("(c p) w -> p c w", p=126),
                us(s)[1:127, 0:4, 1:513])
            nc.sync.dma_start(o[504:512, :], US[16 * s + 1:16 * s + 9, 1:513])

    nc.compile()
    return nc


def _prep_core_inputs(x, f, kernelA, u0):
    """Full [64,...] inputs -> list of 8 per-core input dicts."""
    x = np.asarray(x, np.float32).reshape(B, M, M)
    f = np.asarray(f, np.float32).reshape(B, M, M)
    kern = np.asarray(kernelA, np.float32).reshape(B, 3, 3)
    u0 = np.asarray(u0, np.float32).reshape(B, M, M)

    def pad_full(img, ones_pad):
        P = np.zeros((B, PW, PW), np.float32)
        P[:, 1:513, 1:513] = img
        if ones_pad:
            P[:, 513, :] = 1.0
            P[:, :, 513] = 1.0
        return P

    def chunks5(P):
        Z = np.zeros((B, 128, CH, PW), np.float32)
        for c in range(4):
            Z[:, :, c, :] = P[:, 126 * c:126 * c + 128, :]
        Z[:, 0:10, 4, :] = P[:, 504:514, :]
        return Z

    def chunks4(P):
        Z = np.zeros((B, 128, 4, PW), np.float32)
        for c in range(4):
            Z[:, :, c, :] = P[:, 126 * c:126 * c + 128, :]
        return Z

    Pu = pad_full(u0, True)
    Px = pad_full(x, True)
    Pf = pad_full(f, False)
    u0cL = chunks5(Pu)
    xcL = chunks4(Px)
    fcL = chunks4(Pf)

    # cheb strip: partition 16s+d = sample s P row 504+d
    xsL = np.zeros((NCORES, 128, PW), np.float32)
    fsL = np.zeros((NCORES, 128, PW), np.float32)
    for co in range(NCORES):
        for s in range(SPC):
            xsL[co, 16 * s:16 * s + 10, :] = Px[co * SPC + s, 504:514, :]
            fsL[co, 16 * s + 1:16 * s + 9, :] = Pf[co * SPC + s, 505:513, :]

    # main band: S[k = p-1+a, b*MB+p] = K[a,b], p in 1..126
    wbL = np.zeros((B, 128, WBW), np.float32)
    p_idx = np.arange(1, 127)
    for b in range(3):
        for a in range(3):
            wbL[:, p_idx - 1 + a, b * MB + p_idx] = kern[:, a, b][:, None]
    # per-sample tail band: S[p-1+a, b*9+p] = K[a,b], p in 1..8 (rows 0..9)
    wt5L = np.zeros((B, 128, TBW), np.float32)
    pt_idx = np.arange(1, 9)
    for b in range(3):
        for a in range(3):
            wt5L[:, pt_idx - 1 + a, b * 9 + pt_idx] = kern[:, a, b][:, None]
    # cheb strip band: S[16s+p-1+a, b*MB+16s+p] = K_s[a,b], p in 1..8
    wtsL = np.zeros((NCORES, 128, WBW), np.float32)
    for co in range(NCORES):
        for s in range(SPC):
            ks = kern[co * SPC + s]
            for b in range(3):
                for a in range(3):
                    wtsL[co, 16 * s + pt_idx - 1 + a, b * MB + 16 * s + pt_idx] \
                        = ks[a, b]

    iz = np.zeros((128, 128), np.float32)
    iz[np.arange(1, 127), np.arange(1, 127)] = 1.0
    izs = np.eye(128, dtype=np.float32)
    izL = iz.astype(ml_dtypes.bfloat16)
    izsL = izs.astype(ml_dtypes.bfloat16)

    in_maps = []
    for co in range(NCORES):
        sl = slice(co * SPC, (co + 1) * SPC)
        in_maps.append({
            "u0c": u0cL[sl].transpose(1, 0, 2, 3).reshape(128, SPC * SW).copy(),
            "xc": xcL[sl].transpose(1, 0, 2, 3).reshape(128, SPC * SWF).copy(),
            "fc": fcL[sl].transpose(1, 0, 2, 3).reshape(128, SPC * SWF)
                  .astype(ml_dtypes.bfloat16),
            "xs": xsL[co].copy(),
            "fs": fsL[co].astype(ml_dtypes.bfloat16),
            "wb": wbL[sl].transpose(1, 0, 2).reshape(128, SPC * WBW).copy(),
            "wt5": wt5L[sl].transpose(1, 0, 2).reshape(128, SPC * TBW).copy(),
            "wts": wtsL[co].copy(),
            "iz": izL,
            "izs": izsL,
        })
    return in_maps


def kernel(x, f, kernelA, u0):
    global _COMPILED
    from concourse import bass_utils

    if _COMPILED is None:
        _COMPILED = _build_program()
    nc = _COMPILED

    in_maps = _prep_core_inputs(x, f, kernelA, u0)
    res = bass_utils.run_bass_kernel_spmd(nc, in_maps, core_ids=list(range(NCORES)))
    out = np.stack([res.results[c]["out"] for c in range(NCORES)])  # [8, SPC*M, M]
    return out.reshape(B, 1, M, M).astype(np.float32)
